# revision 1
# baseline (speedup 1.0000x reference)
"""Trainium2 Bass kernel for nn_DQNNetwork (gnn_message_passing).

Reference computation (fp32):
    h  = relu(x @ Wh.T + bh)                       # [n, 512]
    mo = (sum_j h[j] - h) / (n - 1)                # leave-one-out mean
    out = relu(concat([h, mo], 1) @ Wf.T + bf)     # [n, 3] -> flat

Algebraic restructuring (exact up to fp rounding): with Wf = [Wf1 | Wf2],
S = colsum(h), W' = Wf1 - Wf2/(n-1), c = S @ (Wf2.T/(n-1)) + bf:
    out = relu(h @ W'.T + c)
so the only cross-device coupling is S (512 floats) -> one AllReduce.

Sharding: data-parallel over rows. 8 cores x 8192 rows. Weights replicated.

Per-core dataflow (single pass over 16 blocks of 512 rows):
  DMA x block (natural [rows, feat]) -> PE transpose into xT chunks
  -> fp32r GEMM1 (WhT stationary) -> hT in PSUM -> ACT relu+bias(+accum
  colsum) -> SBUF -> fp32r GEMM2 (W'T stationary) -> pre2 [3, rows].
  Tail: AllReduce colsum, c = S @ Wf2T' + bf, out = relu(pre2 + c).

`rep` repeats the whole per-core pipeline (weights loaded once) so wall-clock
deltas between rep values isolate kernel time from the axon RPC overhead.
"""

import numpy as np

import concourse.bacc as bacc
import concourse.mybir as mybir
import concourse.tile as tile
from concourse import bass_utils
from concourse.masks import make_identity

N_CORES = 8
N = 65536               # total rows (stocks)
F = 768                 # input features
H = 512                 # hidden features
A = 3                   # actions
R = N // N_CORES        # rows per core = 8192
RB = 512                # rows per block (fp32 moving-operand max)
NB = R // RB            # blocks per core = 16
NT = RB // 128          # 128-row tiles per block = 4
KF = F // 128           # feature chunks = 6
KH = H // 128           # hidden chunks = 4

F32 = mybir.dt.float32
F32R = mybir.dt.float32r
RELU = mybir.ActivationFunctionType.Relu

_cache = {}


def build_module(rep=1, collective=True, num_devices=N_CORES):
    key = (rep, collective, num_devices)
    if key in _cache:
        return _cache[key]

    nc = bacc.Bacc("TRN2", target_bir_lowering=False, debug=False,
                   num_devices=num_devices)

    x = nc.dram_tensor("x", [R, F], F32R, kind="ExternalInput").ap()
    wht = nc.dram_tensor("wht", [F, H], F32R, kind="ExternalInput").ap()
    bh_t = nc.dram_tensor("bh_t", [128, KH], F32, kind="ExternalInput").ap()
    wpt = nc.dram_tensor("wpt", [128, KH * A], F32R, kind="ExternalInput").ap()
    wf2t = nc.dram_tensor("wf2t", [128, KH * A], F32, kind="ExternalInput").ap()
    bf = nc.dram_tensor("bf", [A, 1], F32, kind="ExternalInput").ap()
    y = nc.dram_tensor("out", [A, R], F32, kind="ExternalOutput").ap()

    x4 = x.rearrange("(b t p) c -> b p t c", t=NT, p=128)  # [16,128,4,768]

    with tile.TileContext(nc) as tc:
        with (
            tc.tile_pool(name="const", bufs=1) as const,
            tc.tile_pool(name="xin", bufs=3) as xin_pool,
            tc.tile_pool(name="xt", bufs=3) as xt_pool,
            tc.tile_pool(name="ht", bufs=2) as ht_pool,
            tc.tile_pool(name="pt", bufs=3, space="PSUM") as pt_pool,
            tc.tile_pool(name="ph", bufs=1, space="PSUM") as ph_pool,
            tc.tile_pool(name="p2", bufs=1, space="PSUM") as p2_pool,
            tc.tile_pool(name="dram", bufs=1, space="DRAM") as dram,
        ):
            ident_f = const.tile([128, 128], F32)
            make_identity(nc, ident_f[:])
            ident = const.tile([128, 128], F32R)
            nc.vector.tensor_copy(ident[:], ident_f[:])   # rounds to f32r
            ident_r = ident[:]

            wht_sb = const.tile([128, KF * H], F32R)
            for k in range(KF):
                nc.scalar.dma_start(out=wht_sb[:, k * H:(k + 1) * H],
                                    in_=wht[k * 128:(k + 1) * 128, :])
            bh_sb = const.tile([128, KH], F32)
            nc.scalar.dma_start(out=bh_sb[:], in_=bh_t[:])
            wpt_sb = const.tile([128, KH * A], F32R)
            nc.scalar.dma_start(out=wpt_sb[:], in_=wpt[:])
            wf2t_sb = const.tile([128, KH * A], F32)
            nc.scalar.dma_start(out=wf2t_sb[:], in_=wf2t[:])
            bf_sb = const.tile([A, 1], F32)
            nc.scalar.dma_start(out=bf_sb[:], in_=bf[:])

            zeros = const.tile([128, RB], F32)
            nc.gpsimd.memset(zeros[:], 0.0)
            s_parts = const.tile([128, KH * NB], F32)   # colsum per (chunk, block)
            pre2 = const.tile([A, R], F32)              # pre-activation of GEMM2
            out_sb = const.tile([A, R], F32)
            s_loc = const.tile([128, KH], F32)
            s_glob = const.tile([128, KH], F32)
            c_sb = const.tile([A, 1], F32)
            s_all = const.tile([128, num_devices * KH], F32)

            for _rep in range(rep):
                pending_g2 = [None]

                def flush_g2():
                    if pending_g2[0] is None:
                        return
                    ht_prev, b_prev = pending_g2[0]
                    pending_g2[0] = None
                    p2 = p2_pool.tile([A, RB], F32, name=f"p2_{b_prev}",
                                      tag="p2")
                    for m in range(KH):
                        nc.tensor.matmul(p2[:], wpt_sb[:, m * A:(m + 1) * A],
                                         ht_prev[:, m * RB:(m + 1) * RB],
                                         start=(m == 0), stop=(m == KH - 1))
                    nc.vector.tensor_copy(
                        pre2[:, b_prev * RB:(b_prev + 1) * RB], p2[:])

                for b in range(NB):
                    x_in = xin_pool.tile([128, NT * F], F32R)
                    if b == 0:
                        # split the first load so transposes start early
                        for t in range(NT):
                            nc.sync.dma_start(
                                out=x_in[:, t * F:(t + 1) * F],
                                in_=x4[b][:, t])
                    else:
                        nc.sync.dma_start(
                            out=x_in[:].rearrange("p (t c) -> p t c", t=NT),
                            in_=x4[b])

                    # transpose x block ([rows, feat] -> xT chunks) with the
                    # k-chunk transposes interleaved ahead of the GEMM1
                    # matmuls that consume them, so PE never waits on copies.
                    xt = xt_pool.tile([128, KF * RB], F32R)
                    ph = [ph_pool.tile([128, RB], F32, tag=f"ph{m}",
                                       name=f"ph{m}_{b}")
                          for m in range(KH)]

                    def do_transpose(k):
                        pt = pt_pool.tile([128, RB], F32R)
                        for t in range(NT):
                            nc.tensor.transpose(
                                pt[:, t * 128:(t + 1) * 128],
                                x_in[:, t * F + k * 128: t * F + (k + 1) * 128],
                                ident_r,
                            )
                        nc.vector.tensor_copy(xt[:, k * RB:(k + 1) * RB],
                                              pt[:])

                    do_transpose(0)
                    do_transpose(1)
                    # prev block's GEMM2 goes here: its relu inputs are ready,
                    # so it never stalls PE at the block boundary
                    flush_g2()
                    for k in range(KF):
                        if k + 2 < KF:
                            do_transpose(k + 2)
                        for m in range(KH):
                            lhs = wht_sb[:, k * H + m * 128:
                                         k * H + (m + 1) * 128]
                            nc.tensor.matmul(ph[m][:], lhs,
                                             xt[:, k * RB:(k + 1) * RB],
                                             start=(k == 0), stop=(k == KF - 1))

                    ht = ht_pool.tile([128, KH * RB], F32R)
                    for m in range(KH):
                        if b == NB - 1 and m % 2 == 1:
                            # last block: relu m=1,3 on DVE so the final
                            # GEMM2 chain is not serialized behind ACT
                            nc.vector.scalar_tensor_tensor(
                                ht[:, m * RB:(m + 1) * RB], ph[m][:],
                                bh_sb[:, m:m + 1], zeros[:],
                                op0=mybir.AluOpType.add,
                                op1=mybir.AluOpType.max,
                                accum_out=s_parts[:, m * NB + b:
                                                  m * NB + b + 1],
                            )
                        else:
                            nc.scalar.activation(
                                ht[:, m * RB:(m + 1) * RB], ph[m][:], RELU,
                                bias=bh_sb[:, m:m + 1],
                                accum_out=s_parts[:, m * NB + b:
                                                  m * NB + b + 1],
                            )
                    pending_g2[0] = (ht, b)

                flush_g2()

                # local colsum: reduce the per-block partials
                nc.vector.tensor_reduce(
                    s_loc[:], s_parts[:].rearrange("p (m b) -> p m b", b=NB),
                    axis=mybir.AxisListType.X, op=mybir.AluOpType.add)

                if collective:
                    # AllGather (floor ~4.6us vs AllReduce ~9.7us) + local sum
                    # (Shared tensors are single-writer: one pair per rep)
                    ar_in = dram.tile([128, KH], F32, name=f"ar_in_{_rep}",
                                      tag=f"ar_in_{_rep}")
                    ag_out = dram.tile([num_devices * 128, KH], F32,
                                       addr_space="Shared",
                                       name=f"ag_out_{_rep}",
                                       tag=f"ag_out_{_rep}")
                    nc.sync.dma_start(out=ar_in[:], in_=s_loc[:])
                    nc.gpsimd.collective_compute(
                        "AllGather", mybir.AluOpType.bypass,
                        replica_groups=[list(range(num_devices))],
                        ins=[ar_in.opt()], outs=[ag_out.opt()],
                    )
                    nc.sync.dma_start(
                        out=s_all[:].rearrange("p (r m) -> p r m", m=KH),
                        in_=ag_out[:].rearrange("(r p) m -> p r m", p=128))
                    nc.vector.tensor_reduce(
                        s_glob[:],
                        s_all[:].rearrange("p (r m) -> p m r", m=KH),
                        axis=mybir.AxisListType.X, op=mybir.AluOpType.add)
                else:
                    nc.vector.tensor_copy(s_glob[:], s_loc[:])

                # c = S @ (Wf2.T/(n-1)) + bf   (tiny fp32 matvec)
                pc = p2_pool.tile([A, RB], F32, tag="p2")
                for m in range(KH):
                    nc.tensor.matmul(pc[:, 0:1], wf2t_sb[:, m * A:(m + 1) * A],
                                     s_glob[:, m:m + 1],
                                     start=(m == 0), stop=(m == KH - 1))
                nc.vector.tensor_add(c_sb[:], pc[:, 0:1], bf_sb[:])

                # out = relu(pre2 + c): split ACT/DVE by engine speed (1.2
                # vs 0.96 GHz), each half's store DMA overlaps the other
                cut = 4608
                nc.scalar.activation(out_sb[:, :cut], pre2[:, :cut], RELU,
                                     bias=c_sb[:])
                nc.vector.tensor_scalar(out_sb[:, cut:], pre2[:, cut:],
                                        scalar1=c_sb[:], scalar2=0.0,
                                        op0=mybir.AluOpType.add,
                                        op1=mybir.AluOpType.max)
                nc.sync.dma_start(out=y[:, :cut], in_=out_sb[:, :cut])
                nc.sync.dma_start(out=y[:, cut:], in_=out_sb[:, cut:])

    nc.compile()
    _cache[key] = nc
    return nc


def prepare_in_maps(x, Wh, bh, Wf, bf):
    x = np.ascontiguousarray(x, dtype=np.float32)
    Wh = np.asarray(Wh, dtype=np.float32)
    bh = np.asarray(bh, dtype=np.float32)
    Wf = np.asarray(Wf, dtype=np.float32)
    bf = np.asarray(bf, dtype=np.float32)

    inv = np.float32(1.0) / np.float32(N - 1)
    Wf1 = Wf[:, :H]
    Wf2s = Wf[:, H:] * inv                      # [3, 512] scaled
    Wp = Wf1 - Wf2s                             # [3, 512]

    def chunk_t(w):                             # [512, 3] -> [128, 12]
        return np.ascontiguousarray(
            w.T.reshape(KH, 128, A).transpose(1, 0, 2).reshape(128, KH * A))

    wht = np.ascontiguousarray(Wh.T)            # [768, 512]
    bh_t = np.ascontiguousarray(bh.reshape(KH, 128).T)  # [128, 4]
    wpt = chunk_t(Wp)
    wf2t = chunk_t(Wf2s)
    bf_c = np.ascontiguousarray(bf.reshape(A, 1))

    shared = {"wht": wht, "bh_t": bh_t, "wpt": wpt, "wf2t": wf2t, "bf": bf_c}
    return [{"x": x[c * R:(c + 1) * R], **shared} for c in range(N_CORES)]


def gather(results):
    full = np.empty((N, A), dtype=np.float32)
    for c, res in enumerate(results):
        full[c * R:(c + 1) * R, :] = res["out"].T
    return full.reshape(-1)


def kernel(x, Wh, bh, Wf, bf):
    nc = build_module()
    in_maps = prepare_in_maps(x, Wh, bh, Wf, bf)
    res = bass_utils.run_bass_kernel_spmd(nc, in_maps,
                                          core_ids=list(range(N_CORES)))
    return gather(res.results)



# revision 2
# speedup vs baseline: 1.9254x; 1.9254x over previous
"""Trainium2 Bass kernel for nn_DQNNetwork (gnn_message_passing).

Reference computation (fp32):
    h  = relu(x @ Wh.T + bh)                       # [n, 512]
    mo = (sum_j h[j] - h) / (n - 1)                # leave-one-out mean
    out = relu(concat([h, mo], 1) @ Wf.T + bf)     # [n, 3] -> flat

Algebraic restructuring (exact up to fp rounding): with Wf = [Wf1 | Wf2],
S = colsum(h), W' = Wf1 - Wf2/(n-1), c = S @ (Wf2.T/(n-1)) + bf:
    out = relu(h @ W'.T + c)
so the only cross-device coupling is c (3 floats) -> one tiny AllGather.

Sharding: data-parallel over rows. 8 cores x 8192 rows. Weights replicated.

v2 layout: the host pre-transposes and packs x into hT-feed order
[128, (block, kchunk, row)] fp16, so the kernel does ZERO on-chip
transposes (saves ~74k PE cycles = 24% of PE time vs v1) and half the
HBM traffic. Per-core dataflow:
  phase A (16 blocks of 512 rows): DMA xT block (one contiguous 6KB/
    partition descriptor) -> fp16 GEMM1 (WhT stationary) -> hT in PSUM
    -> ACT relu+bias, fp32 colsum accum -> hT fp16 kept in SBUF (64KB/p).
  then: colsum reduce -> c_loc = S_loc @ Wf2s.T (tiny PE matvec) ->
    AllGather of 3 floats -> c, all overlapped under phase B's GEMM2.
  phase B (16 blocks): GEMM2 (W'T stationary, hT moving) -> [3, 512]
    PSUM. Early blocks (< CUT, before c lands) buffer pre-activations
    and relu+c on DVE once c arrives; late blocks relu+c directly from
    PSUM on ACT. Output DMA streams per block -> no serial tail.

`rep` repeats the whole per-core pipeline (weights loaded once) so
wall-clock deltas between rep values isolate kernel time from the axon
RPC overhead.
"""

import numpy as np

import concourse.bacc as bacc
import concourse.mybir as mybir
import concourse.tile as tile
from concourse import bass_utils

N_CORES = 8
N = 65536               # total rows (stocks)
F = 768                 # input features
H = 512                 # hidden features
A = 3                   # actions
R = N // N_CORES        # rows per core = 8192
RB = 512                # rows per block
NB = R // RB            # blocks per core = 16
KF = F // 128           # feature chunks = 6
KH = H // 128           # hidden chunks = 4
CUT = 8                 # blocks whose final relu waits for c on DVE

F32 = mybir.dt.float32
F16 = mybir.dt.float16
RELU = mybir.ActivationFunctionType.Relu

_cache = {}


def build_module(rep=1, collective=True, num_devices=N_CORES):
    key = (rep, collective, num_devices)
    if key in _cache:
        return _cache[key]

    nc = bacc.Bacc("TRN2", target_bir_lowering=False, debug=False,
                   num_devices=num_devices)

    x = nc.dram_tensor("x", [128, NB * KF * RB], F16,
                       kind="ExternalInput").ap()
    wht = nc.dram_tensor("wht", [F, H], F16, kind="ExternalInput").ap()
    bh_t = nc.dram_tensor("bh_t", [128, KH], F32, kind="ExternalInput").ap()
    wpt = nc.dram_tensor("wpt", [128, KH * A], F16, kind="ExternalInput").ap()
    wf2t = nc.dram_tensor("wf2t", [128, KH * A], F32,
                          kind="ExternalInput").ap()
    bf = nc.dram_tensor("bf", [A, 1], F32, kind="ExternalInput").ap()
    y = nc.dram_tensor("out", [A, R], F32, kind="ExternalOutput").ap()

    with tile.TileContext(nc) as tc:
        with (
            tc.tile_pool(name="const", bufs=1) as const,
            tc.tile_pool(name="xin", bufs=3) as xin_pool,
            tc.tile_pool(name="ph", bufs=1, space="PSUM") as ph_pool,
            tc.tile_pool(name="p2", bufs=3, space="PSUM") as p2_pool,
            tc.tile_pool(name="dram", bufs=1, space="DRAM") as dram,
        ):
            wht_sb = const.tile([128, KF * H], F16)
            nc.scalar.dma_start(
                out=wht_sb[:].rearrange("p (k h) -> p k h", k=KF),
                in_=wht.rearrange("(k p) h -> p k h", p=128))
            bh_sb = const.tile([128, KH], F32)
            nc.scalar.dma_start(out=bh_sb[:], in_=bh_t[:])
            wpt_sb = const.tile([128, KH * A], F16)
            nc.scalar.dma_start(out=wpt_sb[:], in_=wpt[:])
            wf2t_sb = const.tile([128, KH * A], F32)
            nc.scalar.dma_start(out=wf2t_sb[:], in_=wf2t[:])
            bf_sb = const.tile([A, 1], F32)
            nc.scalar.dma_start(out=bf_sb[:], in_=bf[:])

            ht_all = const.tile([128, KH * R], F16)   # hT, whole shard
            s_parts = const.tile([128, KH * NB], F32)  # colsum per (m, b)
            s_loc = const.tile([128, KH], F32)
            pre2 = const.tile([A, CUT * RB], F32)
            out_sb = const.tile([A, R], F32)
            c_loc = const.tile([A, 1], F32)
            c_all = const.tile([A, num_devices], F32)
            c_red = const.tile([A, 1], F32)
            c_sb = const.tile([A, 1], F32)

            for _rep in range(rep):
                # ---- phase A: GEMM1 + relu(+bias) + colsum accumulation
                for b in range(NB):
                    x_sb = xin_pool.tile([128, KF * RB], F16)
                    nc.sync.dma_start(out=x_sb[:],
                                      in_=x[:, b * KF * RB:(b + 1) * KF * RB])
                    ph = [ph_pool.tile([128, RB], F32, tag=f"ph{m}",
                                       name=f"ph{m}_{b}")
                          for m in range(KH)]
                    for k in range(KF):
                        for m in range(KH):
                            nc.tensor.matmul(
                                ph[m][:],
                                wht_sb[:, k * H + m * 128:
                                       k * H + (m + 1) * 128],
                                x_sb[:, k * RB:(k + 1) * RB],
                                start=(k == 0), stop=(k == KF - 1))
                    for m in range(KH):
                        nc.scalar.activation(
                            ht_all[:, m * R + b * RB:m * R + (b + 1) * RB],
                            ph[m][:], RELU, bias=bh_sb[:, m:m + 1],
                            accum_out=s_parts[:, m * NB + b:m * NB + b + 1])

                # ---- local colsum (DVE) overlaps first GEMM2 blocks on PE
                nc.vector.tensor_reduce(
                    s_loc[:], s_parts[:].rearrange("p (m b) -> p m b", b=NB),
                    axis=mybir.AxisListType.X, op=mybir.AluOpType.add)

                def gemm2(b):
                    p2 = p2_pool.tile([A, RB], F32, name=f"p2_{b}", tag="p2")
                    for m in range(KH):
                        nc.tensor.matmul(
                            p2[:], wpt_sb[:, m * A:(m + 1) * A],
                            ht_all[:, m * R + b * RB:m * R + (b + 1) * RB],
                            start=(m == 0), stop=(m == KH - 1))
                    return p2

                early = [gemm2(b) for b in range(2)]

                # c_loc = S_loc @ (Wf2.T/(n-1)): tiny PE matvec, then the
                # 12-byte AllGather; lands while PE grinds through GEMM2.
                pc = p2_pool.tile([A, RB], F32, tag="p2", name="pc")
                for m in range(KH):
                    nc.tensor.matmul(pc[:, 0:1], wf2t_sb[:, m * A:(m + 1) * A],
                                     s_loc[:, m:m + 1],
                                     start=(m == 0), stop=(m == KH - 1))
                nc.vector.tensor_copy(c_loc[:], pc[:, 0:1])
                if collective:
                    ar_in = dram.tile([A, 1], F32, name=f"ar_in_{_rep}",
                                      tag=f"ar_in_{_rep}")
                    ag_out = dram.tile([num_devices * A, 1], F32,
                                       addr_space="Shared",
                                       name=f"ag_out_{_rep}",
                                       tag=f"ag_out_{_rep}")
                    nc.sync.dma_start(out=ar_in[:], in_=c_loc[:])
                    nc.gpsimd.collective_compute(
                        "AllGather", mybir.AluOpType.bypass,
                        replica_groups=[list(range(num_devices))],
                        ins=[ar_in.opt()], outs=[ag_out.opt()],
                    )
                    nc.sync.dma_start(
                        out=c_all[:],
                        in_=ag_out[:].rearrange("(r a) one -> a (r one)",
                                                a=A))
                    nc.vector.tensor_reduce(
                        c_red[:], c_all[:],
                        axis=mybir.AxisListType.X, op=mybir.AluOpType.add)
                    nc.vector.tensor_add(c_sb[:], c_red[:], bf_sb[:])
                else:
                    nc.vector.tensor_add(c_sb[:], c_loc[:], bf_sb[:])

                # ---- phase B: GEMM2 + relu(+c) + streamed output DMA
                for b, p2 in enumerate(early):
                    nc.vector.tensor_copy(pre2[:, b * RB:(b + 1) * RB],
                                          p2[:])
                for b in range(2, NB):
                    p2 = gemm2(b)
                    if b < CUT:
                        nc.vector.tensor_copy(pre2[:, b * RB:(b + 1) * RB],
                                              p2[:])
                    else:
                        nc.scalar.activation(out_sb[:, b * RB:(b + 1) * RB],
                                             p2[:], RELU, bias=c_sb[:])
                        nc.scalar.dma_start(
                            out=y[:, b * RB:(b + 1) * RB],
                            in_=out_sb[:, b * RB:(b + 1) * RB])
                # early blocks: relu(pre2 + c) on DVE once c arrives
                for b in range(CUT):
                    nc.vector.tensor_scalar(
                        out_sb[:, b * RB:(b + 1) * RB],
                        pre2[:, b * RB:(b + 1) * RB],
                        scalar1=c_sb[:], scalar2=0.0,
                        op0=mybir.AluOpType.add, op1=mybir.AluOpType.max)
                    nc.sync.dma_start(out=y[:, b * RB:(b + 1) * RB],
                                      in_=out_sb[:, b * RB:(b + 1) * RB])

    nc.compile()
    _cache[key] = nc
    return nc


def prepare_in_maps(x, Wh, bh, Wf, bf):
    x = np.asarray(x, dtype=np.float32)
    Wh = np.asarray(Wh, dtype=np.float32)
    bh = np.asarray(bh, dtype=np.float32)
    Wf = np.asarray(Wf, dtype=np.float32)
    bf = np.asarray(bf, dtype=np.float32)

    inv = np.float32(1.0) / np.float32(N - 1)
    Wf1 = Wf[:, :H]
    Wf2s = Wf[:, H:] * inv                      # [3, 512] scaled
    Wp = Wf1 - Wf2s                             # [3, 512]

    def chunk_t(w, dt):                         # [A, 512] -> [128, KH*A]
        return np.ascontiguousarray(
            w.T.reshape(KH, 128, A).transpose(1, 0, 2).reshape(128, KH * A),
            dtype=dt)

    wht = np.ascontiguousarray(Wh.T, dtype=np.float16)       # [768, 512]
    bh_t = np.ascontiguousarray(bh.reshape(KH, 128).T)       # [128, 4]
    wpt = chunk_t(Wp, np.float16)
    wf2t = chunk_t(Wf2s, np.float32)
    bf_c = np.ascontiguousarray(bf.reshape(A, 1))

    shared = {"wht": wht, "bh_t": bh_t, "wpt": wpt, "wf2t": wf2t, "bf": bf_c}

    xh = x.astype(np.float16)
    in_maps = []
    for c in range(N_CORES):
        # pack shard transpose as [128, (block, kchunk, row)]
        xt = xh[c * R:(c + 1) * R].T                  # [768, 8192] view
        xp = np.ascontiguousarray(
            xt.reshape(KF, 128, NB, RB).transpose(1, 2, 0, 3)
              .reshape(128, NB * KF * RB))
        in_maps.append({"x": xp, **shared})
    return in_maps


def gather(results):
    full = np.empty((N, A), dtype=np.float32)
    for c, res in enumerate(results):
        full[c * R:(c + 1) * R, :] = res["out"].T
    return full.reshape(-1)


def kernel(x, Wh, bh, Wf, bf):
    nc = build_module()
    in_maps = prepare_in_maps(x, Wh, bh, Wf, bf)
    res = bass_utils.run_bass_kernel_spmd(nc, in_maps,
                                          core_ids=list(range(N_CORES)))
    return gather(res.results)


# revision 7
# speedup vs baseline: 2.6578x; 1.3804x over previous
"""Trainium2 Bass kernel for nn_DQNNetwork (gnn_message_passing).

Reference computation (fp32):
    h  = relu(x @ Wh.T + bh)                       # [n, 512]
    mo = (sum_j h[j] - h) / (n - 1)                # leave-one-out mean
    out = relu(concat([h, mo], 1) @ Wf.T + bf)     # [n, 3] -> flat

Algebraic restructuring (exact up to fp rounding): with Wf = [Wf1 | Wf2],
S = colsum(h), W' = Wf1 - Wf2/(n-1), c = S @ (Wf2.T/(n-1)) + bf:
    out = relu(h @ W'.T + c)
so the only cross-device coupling is c (3 floats) -> one tiny AllGather.

Sharding: data-parallel over rows. 8 cores x 8192 rows. Weights replicated.

v2 layout: the host pre-transposes and packs x into hT-feed order
[128, (block, kchunk, row)] fp16, so the kernel does ZERO on-chip
transposes (saves ~74k PE cycles = 24% of PE time vs v1) and half the
HBM traffic. Per-core dataflow:
  phase A (16 blocks of 512 rows): DMA xT block (one contiguous 6KB/
    partition descriptor) -> fp16 GEMM1 (WhT stationary) -> hT in PSUM
    -> ACT relu+bias, fp32 colsum accum -> hT fp16 kept in SBUF (64KB/p).
  then: colsum reduce -> c_loc = S_loc @ Wf2s.T (tiny PE matvec) ->
    AllGather of 3 floats -> c, all overlapped under phase B's GEMM2.
  phase B (16 blocks): GEMM2 (W'T stationary, hT moving) -> [3, 512]
    PSUM. Early blocks (< CUT, before c lands) buffer pre-activations
    and relu+c on DVE once c arrives; late blocks relu+c directly from
    PSUM on ACT. Output DMA streams per block -> no serial tail.

`rep` repeats the whole per-core pipeline (weights loaded once) so
wall-clock deltas between rep values isolate kernel time from the axon
RPC overhead.
"""

import numpy as np

import concourse.bacc as bacc
import concourse.mybir as mybir
import concourse.tile as tile
from concourse import bass_utils

N_CORES = 8
N = 65536               # total rows (stocks)
F = 768                 # input features
H = 512                 # hidden features
A = 3                   # actions
R = N // N_CORES        # rows per core = 8192
RB = 512                # rows per block
NB = R // RB            # blocks per core = 16
KF = F // 128           # feature chunks = 6
KH = H // 128           # hidden chunks = 4
CUT = 8                 # blocks whose final relu waits for c on DVE

F32 = mybir.dt.float32
F16 = mybir.dt.float16
RELU = mybir.ActivationFunctionType.Relu

_cache = {}


def build_module(rep=1, collective=True, num_devices=N_CORES):
    key = (rep, collective, num_devices)
    if key in _cache:
        return _cache[key]

    nc = bacc.Bacc("TRN2", target_bir_lowering=False, debug=False,
                   num_devices=num_devices)

    x = nc.dram_tensor("x", [128, NB * KF * RB], F16,
                       kind="ExternalInput").ap()
    wht = nc.dram_tensor("wht", [F, H], F16, kind="ExternalInput").ap()
    bh_t = nc.dram_tensor("bh_t", [128, KH], F32, kind="ExternalInput").ap()
    wpt = nc.dram_tensor("wpt", [128, KH * A], F16, kind="ExternalInput").ap()
    wf2t = nc.dram_tensor("wf2t", [128, KH * A], F32,
                          kind="ExternalInput").ap()
    bf = nc.dram_tensor("bf", [A, 1], F32, kind="ExternalInput").ap()
    y = nc.dram_tensor("out", [A, R], F32, kind="ExternalOutput").ap()

    with tile.TileContext(nc) as tc:
        with (
            tc.tile_pool(name="const", bufs=1) as const,
            tc.tile_pool(name="xin", bufs=3) as xin_pool,
            tc.tile_pool(name="ph", bufs=1, space="PSUM") as ph_pool,
            tc.tile_pool(name="p2", bufs=4, space="PSUM") as p2_pool,
            tc.tile_pool(name="dram", bufs=1, space="DRAM") as dram,
        ):
            wht_sb = const.tile([128, KF * H], F16)
            wht_r = wht.rearrange("(k p) h -> p k h", p=128)
            for k in range(KF):  # per-chunk so cold-start GEMM1 begins early
                nc.scalar.dma_start(out=wht_sb[:, k * H:(k + 1) * H],
                                    in_=wht_r[:, k])
            bh_sb = const.tile([128, KH], F32)
            nc.scalar.dma_start(out=bh_sb[:], in_=bh_t[:])
            wpt_sb = const.tile([128, KH * A], F16)
            nc.scalar.dma_start(out=wpt_sb[:], in_=wpt[:])
            wf2t_sb = const.tile([128, KH * A], F32)
            nc.scalar.dma_start(out=wf2t_sb[:], in_=wf2t[:])
            bf_sb = const.tile([A, 1], F32)
            nc.scalar.dma_start(out=bf_sb[:], in_=bf[:])

            ht_all = const.tile([128, KH * R], F16)   # hT, whole shard
            s_parts = const.tile([128, KH * NB], F32)  # colsum per (m, b)
            s_loc = const.tile([128, KH], F32)
            pre2 = const.tile([A, CUT * RB], F32)
            out_sb = const.tile([A, R], F32)
            c_loc = const.tile([A, 1], F32)
            c_all = const.tile([A, num_devices], F32)
            c_red = const.tile([A, 1], F32)
            c_sb = const.tile([A, 1], F32)

            for _rep in range(rep):
                # ---- phase A: GEMM1 + relu(+bias) + colsum accumulation
                for b in range(NB):
                    x_sb = xin_pool.tile([128, KF * RB], F16)
                    if _rep == 0 and b == 0:
                        # split the cold-start load so GEMM1 starts after
                        # the first k-chunk instead of the whole block
                        for k in range(KF):
                            nc.sync.dma_start(
                                out=x_sb[:, k * RB:(k + 1) * RB],
                                in_=x[:, k * RB:(k + 1) * RB])
                    else:
                        nc.sync.dma_start(
                            out=x_sb[:],
                            in_=x[:, b * KF * RB:(b + 1) * KF * RB])
                    ph = [ph_pool.tile([128, RB], F32, tag=f"ph{m}",
                                       name=f"ph{m}_{b}")
                          for m in range(KH)]
                    for k in range(KF):
                        for m in range(KH):
                            nc.tensor.matmul(
                                ph[m][:],
                                wht_sb[:, k * H + m * 128:
                                       k * H + (m + 1) * 128],
                                x_sb[:, k * RB:(k + 1) * RB],
                                start=(k == 0), stop=(k == KF - 1))
                    for m in range(KH):
                        nc.scalar.activation(
                            ht_all[:, m * R + b * RB:m * R + (b + 1) * RB],
                            ph[m][:], RELU, bias=bh_sb[:, m:m + 1],
                            accum_out=s_parts[:, m * NB + b:m * NB + b + 1])

                # ---- local colsum (DVE) overlaps first GEMM2 blocks on PE
                nc.vector.tensor_reduce(
                    s_loc[:], s_parts[:].rearrange("p (m b) -> p m b", b=NB),
                    axis=mybir.AxisListType.X, op=mybir.AluOpType.add)

                def gemm2(b):
                    p2 = p2_pool.tile([A, RB], F32, name=f"p2_{b}", tag="p2")
                    for m in range(KH):
                        nc.tensor.matmul(
                            p2[:], wpt_sb[:, m * A:(m + 1) * A],
                            ht_all[:, m * R + b * RB:m * R + (b + 1) * RB],
                            start=(m == 0), stop=(m == KH - 1))
                    return p2

                early = [gemm2(b) for b in range(2)]

                # c_loc = S_loc @ (Wf2.T/(n-1)): tiny PE matvec, then the
                # 12-byte AllGather; lands while PE grinds through GEMM2.
                pc = p2_pool.tile([A, RB], F32, tag="p2", name="pc")
                for m in range(KH):
                    nc.tensor.matmul(pc[:, 0:1], wf2t_sb[:, m * A:(m + 1) * A],
                                     s_loc[:, m:m + 1],
                                     start=(m == 0), stop=(m == KH - 1))
                nc.vector.tensor_copy(c_loc[:], pc[:, 0:1])
                if collective:
                    ar_in = dram.tile([A, 1], F32, name=f"ar_in_{_rep}",
                                      tag=f"ar_in_{_rep}")
                    ag_out = dram.tile([num_devices * A, 1], F32,
                                       addr_space="Shared",
                                       name=f"ag_out_{_rep}",
                                       tag=f"ag_out_{_rep}")
                    nc.sync.dma_start(out=ar_in[:], in_=c_loc[:])
                    nc.gpsimd.collective_compute(
                        "AllGather", mybir.AluOpType.bypass,
                        replica_groups=[list(range(num_devices))],
                        ins=[ar_in.opt()], outs=[ag_out.opt()],
                    )
                    nc.sync.dma_start(
                        out=c_all[:],
                        in_=ag_out[:].rearrange("(r a) one -> a (r one)",
                                                a=A))

                # ---- phase B: GEMM2 + relu(+c) + streamed output DMA
                for b, p2 in enumerate(early):
                    nc.vector.tensor_copy(pre2[:, b * RB:(b + 1) * RB],
                                          p2[:])
                for b in range(2, NB):
                    p2 = gemm2(b)
                    if b < CUT:
                        nc.vector.tensor_copy(pre2[:, b * RB:(b + 1) * RB],
                                              p2[:])
                    else:
                        if b == CUT:
                            # c = sum_r c_loc_r + bf. Emitted here (not at
                            # the collective) so the in-order DVE queue
                            # drains the early-block PSUM copies without
                            # waiting on the AllGather.
                            if collective:
                                nc.vector.tensor_reduce(
                                    c_red[:], c_all[:],
                                    axis=mybir.AxisListType.X,
                                    op=mybir.AluOpType.add)
                                nc.vector.tensor_add(c_sb[:], c_red[:],
                                                     bf_sb[:])
                            else:
                                nc.vector.tensor_add(c_sb[:], c_loc[:],
                                                     bf_sb[:])
                        nc.scalar.activation(out_sb[:, b * RB:(b + 1) * RB],
                                             p2[:], RELU, bias=c_sb[:])
                        nc.scalar.dma_start(
                            out=y[:, b * RB:(b + 1) * RB],
                            in_=out_sb[:, b * RB:(b + 1) * RB])
                # early blocks: relu(pre2 + c) on DVE once c arrives
                for b in range(CUT):
                    nc.vector.tensor_scalar(
                        out_sb[:, b * RB:(b + 1) * RB],
                        pre2[:, b * RB:(b + 1) * RB],
                        scalar1=c_sb[:], scalar2=0.0,
                        op0=mybir.AluOpType.add, op1=mybir.AluOpType.max)
                    nc.sync.dma_start(out=y[:, b * RB:(b + 1) * RB],
                                      in_=out_sb[:, b * RB:(b + 1) * RB])

    nc.compile()
    _cache[key] = nc
    return nc


def prepare_in_maps(x, Wh, bh, Wf, bf):
    x = np.asarray(x, dtype=np.float32)
    Wh = np.asarray(Wh, dtype=np.float32)
    bh = np.asarray(bh, dtype=np.float32)
    Wf = np.asarray(Wf, dtype=np.float32)
    bf = np.asarray(bf, dtype=np.float32)

    inv = np.float32(1.0) / np.float32(N - 1)
    Wf1 = Wf[:, :H]
    Wf2s = Wf[:, H:] * inv                      # [3, 512] scaled
    Wp = Wf1 - Wf2s                             # [3, 512]

    def chunk_t(w, dt):                         # [A, 512] -> [128, KH*A]
        return np.ascontiguousarray(
            w.T.reshape(KH, 128, A).transpose(1, 0, 2).reshape(128, KH * A),
            dtype=dt)

    wht = np.ascontiguousarray(Wh.T, dtype=np.float16)       # [768, 512]
    bh_t = np.ascontiguousarray(bh.reshape(KH, 128).T)       # [128, 4]
    wpt = chunk_t(Wp, np.float16)
    wf2t = chunk_t(Wf2s, np.float32)
    bf_c = np.ascontiguousarray(bf.reshape(A, 1))

    shared = {"wht": wht, "bh_t": bh_t, "wpt": wpt, "wf2t": wf2t, "bf": bf_c}

    xh = x.astype(np.float16)
    in_maps = []
    for c in range(N_CORES):
        # pack shard transpose as [128, (block, kchunk, row)]
        xt = xh[c * R:(c + 1) * R].T                  # [768, 8192] view
        xp = np.ascontiguousarray(
            xt.reshape(KF, 128, NB, RB).transpose(1, 2, 0, 3)
              .reshape(128, NB * KF * RB))
        in_maps.append({"x": xp, **shared})
    return in_maps


def gather(results):
    full = np.empty((N, A), dtype=np.float32)
    for c, res in enumerate(results):
        full[c * R:(c + 1) * R, :] = res["out"].T
    return full.reshape(-1)


def kernel(x, Wh, bh, Wf, bf):
    nc = build_module()
    in_maps = prepare_in_maps(x, Wh, bh, Wf, bf)
    res = bass_utils.run_bass_kernel_spmd(nc, in_maps,
                                          core_ids=list(range(N_CORES)))
    return gather(res.results)
